# revision 11
# baseline (speedup 1.0000x reference)
"""CharRNN Trainium2 kernel.

Data-parallel over batch: B=64 split as 8 sequences per NeuronCore.
Per core, the whole problem is SBUF-resident:
  - xw_t = embed(x_t) @ Wxh + bh is folded host-side into a table
    G = embed_W @ Wxh + bh  (V=128, H=1024); per step the row-gather is
    done on the PE as a one-hot matmul that initializes the PSUM
    accumulator (start=True), so the recurrence matmuls accumulate on top.
  - recurrence: z_t = xw_t + h_{t-1} @ Whh with lhsT = hT (stationary,
    8 cols) and Whh as the moving operand; tanh on ScalarE.
  - hT maintained with PE transposes (h is batch-major after tanh).
  - logits_t = lin_W @ h_t computed v-major (lin_WT stationary), written
    into an SBUF-resident (V, T*B_L) buffer, DMA'd out once at the end.
All matmul inputs are bf16 (fp32 PSUM accumulation); measured end-to-end
relative error vs an fp64 reference is ~3.5e-3.
"""

import numpy as np
import ml_dtypes

import concourse.bass as bass
import concourse.mybir as mybir
import concourse.tile as _tile_mod
from concourse.tile import TileContext
from concourse.bass_utils import run_bass_kernel_spmd
from concourse.vector_clock import ScopedClock as _ScopedClock


def _patched_drain_and_barrier(self, tick_clock, wait_clock):
    """The stock kernel-tail Drain carries one sync-wait per outstanding
    proc; the walrus build in this container only accepts a couple of
    sync waits per CTRL instruction ("Too many sync wait commands").
    Split the waits into standalone wait_ge instructions on SP, then a
    bare drain. Semantics are identical: SP is in-order, so all waits
    retire before the drain executes."""
    nc = self.nc
    probe = nc.sync.drain()
    wait_clock.add_sem_waits(probe.ins, _ScopedClock({None: tick_clock.global_clock}))
    si = probe.ins.sync_info
    waits = list(si.on_wait) if si is not None else []
    if len(waits) > 1:
        assert self.sems is not None
        by_name = {h.name: h for h in self.sems.allocated().values()}
        kept = []
        for w in waits:
            h = by_name.get(w.ant_name)
            if (h is not None and w.wait_reg is None
                    and w.wait_mode == "sem-ge-imm"):
                nc.sync.wait_ge(h, w.wait_value)
            else:
                kept.append(w)
        si.on_wait = kept
        # re-emit the drain AFTER the waits; neutralize the probe into a
        # pure drain that only carries unresolvable waits
        nc.sync.drain()

    nc.all_engine_barrier()
    assert self.sems is not None
    popped = nc._tile_sem_poison_stack.pop()
    assert popped is self._sem_poison
    nc.clear_and_free_semaphores(list(self.sems.allocated().values()))
    nc.all_engine_barrier()


_tile_mod.TileContext._drain_and_barrier = _patched_drain_and_barrier

# Walrus in this container caps sync-wait commands per instruction. Tile's
# add_semaphores can attach more. Spill excess waits onto preceding NOPs on
# the same engine (engines are in-order, so semantics are preserved).
_MAX_WAITS = int(__import__("os").environ.get("BASS_MAX_WAITS", "1"))
_orig_commit = _tile_mod.TileContext._commit_instruction


def _patched_commit(self, inst, lazy_reg_writes=True):
    si = getattr(inst, "sync_info", None)
    eng = getattr(inst, "engine", None)
    if (
        si is not None
        and eng is not None
        and eng != mybir.EngineType.Unassigned
        and len(si.on_wait) > _MAX_WAITS
    ):
        waits = list(si.on_wait)
        excess, keep = waits[:-_MAX_WAITS], waits[-_MAX_WAITS:]
        for i in range(0, len(excess), _MAX_WAITS):
            nop = self.nc.engines[eng].nop()
            nsi = nop.ins.sync_info
            if nsi is None:
                nop.ins.sync_info = type(si)(
                    on_wait=excess[i:i + _MAX_WAITS], on_update=[]
                )
            else:
                nsi.on_wait = excess[i:i + _MAX_WAITS]
        si.on_wait = keep
    return _orig_commit(self, inst, lazy_reg_writes)


_tile_mod.TileContext._commit_instruction = _patched_commit

VOCAB = 128
EMBED = 256
HIDDEN = 1024
B_FULL = 64
T_FULL = 1024
NCORES = 8
BL = B_FULL // NCORES          # 8 sequences per core
KT = HIDDEN // 128             # 8 k-tiles over the hidden dim
NH = HIDDEN // 512             # 2 psum-bank halves of z
U = 8                          # steps per loop-block (even)

BF16 = mybir.dt.bfloat16
F32 = mybir.dt.float32
TANH = mybir.ActivationFunctionType.Tanh


def build_nc(T=T_FULL, debug=False):
    assert T % U == 0 and T >= U
    nblk = T // U

    nc = bass.Bass("TRN2", target_bir_lowering=False, debug=debug)

    oneh_hbm = nc.dram_tensor("oneh", [VOCAB, T * BL], BF16, kind="ExternalInput")
    g_hbm = nc.dram_tensor("g", [VOCAB, HIDDEN], BF16, kind="ExternalInput")
    whh_hbm = nc.dram_tensor("whh", [128, KT * HIDDEN], BF16, kind="ExternalInput")
    linw_hbm = nc.dram_tensor("linw", [128, KT * VOCAB], BF16, kind="ExternalInput")
    ident_hbm = nc.dram_tensor("ident", [BL, BL], BF16, kind="ExternalInput")
    logitsT_hbm = nc.dram_tensor("logitsT", [VOCAB, T * BL], F32, kind="ExternalOutput")
    hlast_hbm = nc.dram_tensor("hlast", [BL, HIDDEN], F32, kind="ExternalOutput")

    with TileContext(nc) as tc:
        with (
            tc.tile_pool(name="const", bufs=1) as cpool,
            tc.tile_pool(name="state", bufs=1) as spool,
            tc.tile_pool(name="pz", bufs=2, space="PSUM") as pzp,
            tc.tile_pool(name="pt", bufs=2, space="PSUM") as ptp,
            tc.tile_pool(name="pl", bufs=2, space="PSUM") as plp,
        ):
            oneh_sb = cpool.tile([VOCAB, T * BL], BF16, tag="oneh")
            g_sb = cpool.tile([VOCAB, HIDDEN], BF16, tag="g")
            whh_sb = cpool.tile([128, KT * HIDDEN], BF16, tag="whh")
            linw_sb = cpool.tile([128, KT * VOCAB], BF16, tag="linw")
            ident_sb = cpool.tile([BL, BL], BF16, tag="ident")
            logitsT_sb = cpool.tile([VOCAB, T * BL], F32, tag="lgt")
            hlast_sb = cpool.tile([BL, HIDDEN], F32, tag="hlast")

            nc.sync.dma_start(oneh_sb[:], oneh_hbm[:])
            nc.sync.dma_start(g_sb[:], g_hbm[:])
            nc.sync.dma_start(whh_sb[:], whh_hbm[:])
            nc.sync.dma_start(linw_sb[:], linw_hbm[:])
            nc.sync.dma_start(ident_sb[:], ident_hbm[:])

            hT = [spool.tile([128, KT * BL], BF16, tag=f"hT{p}", name=f"hT{p}") for p in range(2)]
            hbm = [spool.tile([BL, HIDDEN], BF16, tag=f"hbm{p}", name=f"hbm{p}") for p in range(2)]
            # block staging: one dynamic DVE copy per loop body keeps the
            # register pressure O(1) (each snap pins a register per engine)
            ohblk = spool.tile([VOCAB, U * BL], BF16, tag="ohblk", name="ohblk")
            lgblk = spool.tile([VOCAB, U * BL], F32, tag="lgblk", name="lgblk")

            def emit_logits(u_prev, dst):
                # logits for step t-1, v-major: logitsT = sum_k lin_WT_k.T @ hT_k
                hT_prev = hT[(u_prev + 1) % 2]  # written at step u_prev
                pl = plp.tile([VOCAB, BL], F32, tag="pl", name="pl")
                for k in range(KT):
                    nc.tensor.matmul(
                        pl[:],
                        linw_sb[:, k * VOCAB:(k + 1) * VOCAB],
                        hT_prev[:, k * BL:(k + 1) * BL],
                        start=(k == 0),
                        stop=(k == KT - 1),
                    )
                nc.vector.tensor_copy(dst, pl[:])

            def emit_step(u, oneh_lhsT, lg_dst, first, last):
                # z = xw + h @ Whh, tanh, maintain hT; logits of the
                # PREVIOUS step are emitted between so they hide the tanh
                # tail on the PE.
                hT_prev = hT[u % 2]
                hT_next = hT[(u + 1) % 2]
                hb = hbm[u % 2]
                pz = pzp.tile([BL, HIDDEN], F32, tag="pz", name="pz")
                for nh in range(NH):
                    zsl = pz[:, nh * 512:(nh + 1) * 512]
                    nc.tensor.matmul(
                        zsl, oneh_lhsT, g_sb[:, nh * 512:(nh + 1) * 512],
                        start=True, stop=first,
                    )
                    if not first:
                        for k in range(KT):
                            nc.tensor.matmul(
                                zsl,
                                hT_prev[:, k * BL:(k + 1) * BL],
                                whh_sb[:, k * HIDDEN + nh * 512:
                                       k * HIDDEN + nh * 512 + 512],
                                start=False, stop=(k == KT - 1),
                            )
                if not first:
                    emit_logits(u - 1, lg_dst)
                for nh in range(NH):
                    nc.scalar.activation(
                        hb[:, nh * 512:(nh + 1) * 512],
                        pz[:, nh * 512:(nh + 1) * 512], TANH,
                    )
                if last:
                    for nh in range(NH):
                        nc.scalar.activation(
                            hlast_sb[:, nh * 512:(nh + 1) * 512],
                            pz[:, nh * 512:(nh + 1) * 512], TANH,
                        )
                pt = ptp.tile([128, KT * BL], BF16, tag="pt", name="pt")
                for k in range(KT):
                    nc.tensor.transpose(
                        pt[:, k * BL:(k + 1) * BL],
                        hb[:, k * 128:(k + 1) * 128],
                        ident_sb[:],
                    )
                nc.vector.tensor_copy(hT_next[:], pt[:])

            # block 0 (t = 0..U-1), static
            for u in range(U):
                t = u
                lg_dst = logitsT_sb[:, (t - 1) * BL:t * BL]
                emit_step(u, oneh_sb[:, t * BL:(t + 1) * BL], lg_dst,
                          first=(u == 0), last=(nblk == 1 and u == U - 1))

            # blocks 1..nblk-2, hardware loop. All per-step slices are
            # static; the only dynamic accesses are two whole-block DVE
            # copies (one-hot stage in, logits flush out).
            if nblk > 2:
                with tc.For_i(1, nblk - 1) as i:
                    tb = nc.snap(i * (U * BL))
                    tbm1 = nc.snap(tb - BL)
                    nc.vector.tensor_copy(ohblk[:], oneh_sb[:, bass.ds(tb, U * BL)])
                    for u in range(U):
                        emit_step(u, ohblk[:, u * BL:(u + 1) * BL],
                                  lgblk[:, u * BL:(u + 1) * BL],
                                  first=False, last=False)
                    # flush logits of steps [i*U-1, i*U+U-2]
                    nc.vector.tensor_copy(logitsT_sb[:, bass.ds(tbm1, U * BL)], lgblk[:])

            # last block, static
            if nblk > 1:
                t0 = (nblk - 1) * U
                for u in range(U):
                    t = t0 + u
                    lg_dst = logitsT_sb[:, (t - 1) * BL:t * BL]
                    emit_step(u, oneh_sb[:, t * BL:(t + 1) * BL], lg_dst,
                              first=False, last=(u == U - 1))

            # trailing logits for the final step
            emit_logits(U - 1, logitsT_sb[:, (T - 1) * BL:T * BL])

            nc.sync.dma_start(logitsT_hbm[:], logitsT_sb[:])
            nc.sync.dma_start(hlast_hbm[:], hlast_sb[:])

    return nc


def prep_inputs(x, embed_W, Wxh, Whh, bh, lin_W, T=T_FULL):
    """Host-side prep: fold embed@Wxh+bh into G, build per-core one-hots,
    lay out weights for SBUF residency. Returns list of per-core in_maps."""
    x = np.asarray(x)
    f = np.float32
    G = (np.asarray(embed_W, f) @ np.asarray(Wxh, f) + np.asarray(bh, f))
    g_bf = G.astype(ml_dtypes.bfloat16)
    whh = np.asarray(Whh, f).reshape(KT, 128, HIDDEN).transpose(1, 0, 2)
    whh_bf = np.ascontiguousarray(whh).reshape(128, KT * HIDDEN).astype(ml_dtypes.bfloat16)
    linwT = np.asarray(lin_W, f).T.reshape(KT, 128, VOCAB).transpose(1, 0, 2)
    linw_bf = np.ascontiguousarray(linwT).reshape(128, KT * VOCAB).astype(ml_dtypes.bfloat16)
    ident = np.eye(BL, dtype=ml_dtypes.bfloat16)

    in_maps = []
    for c in range(NCORES):
        xc = x[c * BL:(c + 1) * BL, :T].astype(np.int64)  # (BL, T)
        oneh = np.zeros((VOCAB, T, BL), dtype=ml_dtypes.bfloat16)
        tt, bb = np.meshgrid(np.arange(T), np.arange(BL), indexing="ij")
        oneh[xc.T[tt, bb], tt, bb] = 1
        in_maps.append({
            "oneh": np.ascontiguousarray(oneh).reshape(VOCAB, T * BL),
            "g": g_bf,
            "whh": whh_bf,
            "linw": linw_bf,
            "ident": ident,
        })
    return in_maps


_NC_CACHE = {}


def run_cores(x, embed_W, Wxh, Whh, bh, lin_W, T=T_FULL, trace=False):
    if T not in _NC_CACHE:
        _NC_CACHE[T] = build_nc(T)
    nc = _NC_CACHE[T]
    in_maps = prep_inputs(x, embed_W, Wxh, Whh, bh, lin_W, T=T)
    res = run_bass_kernel_spmd(nc, in_maps, list(range(NCORES)), trace=trace)
    logits = np.empty((B_FULL, T, VOCAB), np.float32)
    h_last = np.empty((B_FULL, HIDDEN), np.float32)
    for c in range(NCORES):
        lt = res.results[c]["logitsT"].reshape(VOCAB, T, BL)
        logits[c * BL:(c + 1) * BL] = lt.transpose(2, 1, 0)
        h_last[c * BL:(c + 1) * BL] = res.results[c]["hlast"]
    return logits, h_last, res


def kernel(x, embed_W, Wxh, Whh, bh, lin_W, lin_b):
    logits, h_last, _ = run_cores(x, embed_W, Wxh, Whh, bh, lin_W, T=T_FULL)
    logits = logits + np.asarray(lin_b, np.float32)[None, None, :]
    return logits, h_last


# revision 14
# speedup vs baseline: 1.2593x; 1.2593x over previous
"""CharRNN Trainium2 kernel.

Data-parallel over batch: B=64 split as 8 sequences per NeuronCore.
Per core, the whole problem is SBUF-resident:
  - xw_t = embed(x_t) @ Wxh + bh is folded host-side into a table
    G = embed_W @ Wxh + bh  (V=128, H=1024); per step the row-gather is
    done on the PE as a one-hot matmul that initializes the PSUM
    accumulator (start=True), so the recurrence matmuls accumulate on top.
  - recurrence: z_t = xw_t + h_{t-1} @ Whh with lhsT = hT (stationary,
    8 cols) and Whh as the moving operand; tanh on ScalarE.
  - hT maintained with PE transposes (h is batch-major after tanh).
  - logits_t = lin_W @ h_t computed v-major (lin_WT stationary), written
    into an SBUF-resident (V, T*B_L) buffer, DMA'd out once at the end.
All matmul inputs are bf16 (fp32 PSUM accumulation); measured end-to-end
relative error vs an fp64 reference is ~3.5e-3.
"""

import numpy as np
import ml_dtypes

import concourse.bass as bass
import concourse.mybir as mybir
import concourse.tile as _tile_mod
from concourse.tile import TileContext
from concourse.bass_utils import run_bass_kernel_spmd
from concourse.vector_clock import ScopedClock as _ScopedClock


def _patched_drain_and_barrier(self, tick_clock, wait_clock):
    """The stock kernel-tail Drain carries one sync-wait per outstanding
    proc; the walrus build in this container only accepts a couple of
    sync waits per CTRL instruction ("Too many sync wait commands").
    Split the waits into standalone wait_ge instructions on SP, then a
    bare drain. Semantics are identical: SP is in-order, so all waits
    retire before the drain executes."""
    nc = self.nc
    probe = nc.sync.drain()
    wait_clock.add_sem_waits(probe.ins, _ScopedClock({None: tick_clock.global_clock}))
    si = probe.ins.sync_info
    waits = list(si.on_wait) if si is not None else []
    if len(waits) > 1:
        assert self.sems is not None
        by_name = {h.name: h for h in self.sems.allocated().values()}
        kept = []
        for w in waits:
            h = by_name.get(w.ant_name)
            if (h is not None and w.wait_reg is None
                    and w.wait_mode == "sem-ge-imm"):
                nc.sync.wait_ge(h, w.wait_value)
            else:
                kept.append(w)
        si.on_wait = kept
        # re-emit the drain AFTER the waits; neutralize the probe into a
        # pure drain that only carries unresolvable waits
        nc.sync.drain()

    nc.all_engine_barrier()
    assert self.sems is not None
    popped = nc._tile_sem_poison_stack.pop()
    assert popped is self._sem_poison
    nc.clear_and_free_semaphores(list(self.sems.allocated().values()))
    nc.all_engine_barrier()


_tile_mod.TileContext._drain_and_barrier = _patched_drain_and_barrier

# Walrus in this container caps sync-wait commands per instruction. Tile's
# add_semaphores can attach more. Spill excess waits onto preceding NOPs on
# the same engine (engines are in-order, so semantics are preserved).
_MAX_WAITS = int(__import__("os").environ.get("BASS_MAX_WAITS", "1"))
_orig_commit = _tile_mod.TileContext._commit_instruction


def _patched_commit(self, inst, lazy_reg_writes=True):
    si = getattr(inst, "sync_info", None)
    eng = getattr(inst, "engine", None)
    if (
        si is not None
        and eng is not None
        and eng != mybir.EngineType.Unassigned
        and len(si.on_wait) > _MAX_WAITS
    ):
        waits = list(si.on_wait)
        excess, keep = waits[:-_MAX_WAITS], waits[-_MAX_WAITS:]
        for i in range(0, len(excess), _MAX_WAITS):
            nop = self.nc.engines[eng].nop()
            nsi = nop.ins.sync_info
            if nsi is None:
                nop.ins.sync_info = type(si)(
                    on_wait=excess[i:i + _MAX_WAITS], on_update=[]
                )
            else:
                nsi.on_wait = excess[i:i + _MAX_WAITS]
        si.on_wait = keep
    return _orig_commit(self, inst, lazy_reg_writes)


_tile_mod.TileContext._commit_instruction = _patched_commit

VOCAB = 128
EMBED = 256
HIDDEN = 1024
B_FULL = 64
T_FULL = 1024
NCORES = 8
BL = B_FULL // NCORES          # 8 sequences per core
KT = HIDDEN // 128             # 8 k-tiles over the hidden dim
NH = HIDDEN // 512             # 2 psum-bank halves of z
U = 32                         # steps per loop-block (even)

HINT_ENGINES = (mybir.EngineType.PE,)
BF16 = mybir.dt.bfloat16
F32 = mybir.dt.float32
TANH = mybir.ActivationFunctionType.Tanh


def build_nc(T=T_FULL, debug=False, staggered=False):
    assert T % U == 0 and T >= U
    nblk = T // U

    nc = bass.Bass("TRN2", target_bir_lowering=False, debug=debug)

    oneh_hbm = nc.dram_tensor("oneh", [VOCAB, T * BL], BF16, kind="ExternalInput")
    g_hbm = nc.dram_tensor("g", [VOCAB, HIDDEN], BF16, kind="ExternalInput")
    whh_hbm = nc.dram_tensor("whh", [128, KT * HIDDEN], BF16, kind="ExternalInput")
    linw_hbm = nc.dram_tensor("linw", [128, KT * VOCAB], BF16, kind="ExternalInput")
    ident_hbm = nc.dram_tensor("ident", [BL, BL], BF16, kind="ExternalInput")
    logitsT_hbm = nc.dram_tensor("logitsT", [VOCAB, T * BL], F32, kind="ExternalOutput")
    hlast_hbm = nc.dram_tensor("hlast", [BL, HIDDEN], F32, kind="ExternalOutput")

    with TileContext(nc) as tc:
        with (
            tc.tile_pool(name="const", bufs=1) as cpool,
            tc.tile_pool(name="state", bufs=1) as spool,
            tc.tile_pool(name="pzp", bufs=1, space="PSUM") as pzp,
            tc.tile_pool(name="pt", bufs=2, space="PSUM") as ptp,
            tc.tile_pool(name="pl", bufs=2, space="PSUM") as plp,
        ):
            oneh_sb = cpool.tile([VOCAB, T * BL], BF16, tag="oneh")
            g_sb = cpool.tile([VOCAB, HIDDEN], BF16, tag="g")
            whh_sb = cpool.tile([128, KT * HIDDEN], BF16, tag="whh")
            linw_sb = cpool.tile([128, KT * VOCAB], BF16, tag="linw")
            ident_sb = cpool.tile([BL, BL], BF16, tag="ident")
            logitsT_sb = cpool.tile([VOCAB, T * BL], F32, tag="lgt")
            hlast_sb = cpool.tile([BL, HIDDEN], F32, tag="hlast")

            nc.sync.dma_start(oneh_sb[:], oneh_hbm[:])
            nc.sync.dma_start(g_sb[:], g_hbm[:])
            nc.sync.dma_start(whh_sb[:], whh_hbm[:])
            nc.sync.dma_start(linw_sb[:], linw_hbm[:])
            nc.sync.dma_start(ident_sb[:], ident_hbm[:])

            hT = [spool.tile([128, KT * BL], BF16, tag=f"hT{p}", name=f"hT{p}") for p in range(2)]
            hbm = [spool.tile([BL, HIDDEN], BF16, tag=f"hbm{p}", name=f"hbm{p}") for p in range(2)]
            # z PSUM accumulators: per-bank, ping-pong across steps. The xw
            # one-hot matmul for step t+1 initializes the bank (start=True)
            # already during step t, and tanh of a bank starts as soon as
            # that bank's k-loop is done.
            pz = [[pzp.tile([BL, 512], F32, tag=f"pz{nh}{p}", name=f"pz{nh}{p}")
                   for p in range(2)] for nh in range(NH)]
            # block staging: one dynamic DVE copy per loop body keeps the
            # register pressure O(1) (each snap pins a register per engine)
            ohblk = spool.tile([VOCAB, U * BL], BF16, tag="ohblk", name="ohblk")
            ohnxt = spool.tile([VOCAB, BL], BF16, tag="ohnxt", name="ohnxt")
            lgblk = spool.tile([VOCAB, U * BL], F32, tag="lgblk", name="lgblk")

            def emit_xw(t_parity, oneh_lhsT, first=False):
                # initialize both z banks for a step with the xw one-hot MM
                for nh in range(NH):
                    nc.tensor.matmul(
                        pz[nh][t_parity][:], oneh_lhsT,
                        g_sb[:, nh * 512:(nh + 1) * 512],
                        start=True, stop=first, skip_group_check=True,
                    )

            def emit_logits(u_prev, dst):
                # logits for step t-1, v-major: logitsT = sum_k lin_WT_k.T @ hT_k
                hT_prev = hT[(u_prev + 1) % 2]  # written at step u_prev
                pl = plp.tile([VOCAB, BL], F32, tag="pl", name="pl")
                for k in range(KT):
                    nc.tensor.matmul(
                        pl[:],
                        linw_sb[:, k * VOCAB:(k + 1) * VOCAB],
                        hT_prev[:, k * BL:(k + 1) * BL],
                        start=(k == 0),
                        stop=(k == KT - 1),
                    )
                nc.vector.tensor_copy(dst, pl[:])

            def emit_step(u, oneh_next, lg_dst, first, last):
                # One recurrence step. xw for this step is already in PSUM
                # (emitted a step earlier); this emits the k-loop, logits of
                # the previous step, xw of the NEXT step (fills the PE while
                # tanh runs), then tanh/transpose/copy split per z-bank so
                # dependent work starts as early as possible.
                p = u % 2
                hT_prev = hT[p]
                hT_next = hT[(u + 1) % 2]
                hb = hbm[p]
                if not first:
                    for nh in range(NH):
                        for k in range(KT):
                            nc.tensor.matmul(
                                pz[nh][p][:],
                                hT_prev[:, k * BL:(k + 1) * BL],
                                whh_sb[:, k * HIDDEN + nh * 512:
                                       k * HIDDEN + nh * 512 + 512],
                                start=False, stop=(k == KT - 1),
                                skip_group_check=True,
                            )
                    emit_logits(u - 1, lg_dst)
                if oneh_next is not None:
                    emit_xw((u + 1) % 2, oneh_next)
                pt = ptp.tile([128, KT * BL], BF16, tag="pt", name="pt")
                for nh in range(NH):
                    nc.scalar.activation(
                        hb[:, nh * 512:(nh + 1) * 512],
                        pz[nh][p][:], TANH,
                    )
                    if last:
                        nc.scalar.activation(
                            hlast_sb[:, nh * 512:(nh + 1) * 512],
                            pz[nh][p][:], TANH,
                        )
                    for k in range(nh * 4, nh * 4 + 4):
                        nc.tensor.transpose(
                            pt[:, k * BL:(k + 1) * BL],
                            hb[:, k * 128:(k + 1) * 128],
                            ident_sb[:],
                        )
                    half = slice(nh * 4 * BL, (nh * 4 + 4) * BL)
                    nc.vector.tensor_copy(hT_next[:, half], pt[:, half])

            # block 0 (t = 0..U-1), static
            emit_xw(0, oneh_sb[:, 0:BL], first=True)
            for u in range(U):
                t = u
                lg_dst = logitsT_sb[:, (t - 1) * BL:t * BL]
                nxt = oneh_sb[:, (t + 1) * BL:(t + 2) * BL] if nblk > 1 or u < U - 1 else None
                emit_step(u, nxt, lg_dst, first=(u == 0),
                          last=(nblk == 1 and u == U - 1))

            # blocks 1..nblk-2, hardware loop. All per-step slices are
            # static; the only dynamic accesses are three DVE copies per
            # body (one-hot block stage in, next-block first one-hot for
            # the xw-ahead of the last step, logits flush out).
            if nblk > 2:
                with tc.For_i(1, nblk - 1, hint_engines=HINT_ENGINES,
                              staggered_reset=staggered) as i:
                    tb = nc.snap(i * (U * BL))
                    tbm1 = nc.snap(tb - BL)
                    tbn = nc.snap(tb + U * BL)
                    nc.vector.tensor_copy(ohblk[:], oneh_sb[:, bass.ds(tb, U * BL)])
                    nc.vector.tensor_copy(ohnxt[:], oneh_sb[:, bass.ds(tbn, BL)])
                    for u in range(U):
                        if u < U - 1:
                            nxt = ohblk[:, (u + 1) * BL:(u + 2) * BL]
                        else:
                            nxt = ohnxt[:]
                        emit_step(u, nxt, lgblk[:, u * BL:(u + 1) * BL],
                                  first=False, last=False)
                    # flush logits of steps [i*U-1, i*U+U-2]
                    nc.vector.tensor_copy(logitsT_sb[:, bass.ds(tbm1, U * BL)], lgblk[:])

            # last block, static
            if nblk > 1:
                t0 = (nblk - 1) * U
                for u in range(U):
                    t = t0 + u
                    lg_dst = logitsT_sb[:, (t - 1) * BL:t * BL]
                    nxt = oneh_sb[:, (t + 1) * BL:(t + 2) * BL] if u < U - 1 else None
                    emit_step(u, nxt, lg_dst, first=False, last=(u == U - 1))

            # trailing logits for the final step
            emit_logits(U - 1, logitsT_sb[:, (T - 1) * BL:T * BL])

            nc.sync.dma_start(logitsT_hbm[:], logitsT_sb[:])
            nc.sync.dma_start(hlast_hbm[:], hlast_sb[:])

    return nc


def prep_inputs(x, embed_W, Wxh, Whh, bh, lin_W, T=T_FULL):
    """Host-side prep: fold embed@Wxh+bh into G, build per-core one-hots,
    lay out weights for SBUF residency. Returns list of per-core in_maps."""
    x = np.asarray(x)
    f = np.float32
    G = (np.asarray(embed_W, f) @ np.asarray(Wxh, f) + np.asarray(bh, f))
    g_bf = G.astype(ml_dtypes.bfloat16)
    whh = np.asarray(Whh, f).reshape(KT, 128, HIDDEN).transpose(1, 0, 2)
    whh_bf = np.ascontiguousarray(whh).reshape(128, KT * HIDDEN).astype(ml_dtypes.bfloat16)
    linwT = np.asarray(lin_W, f).T.reshape(KT, 128, VOCAB).transpose(1, 0, 2)
    linw_bf = np.ascontiguousarray(linwT).reshape(128, KT * VOCAB).astype(ml_dtypes.bfloat16)
    ident = np.eye(BL, dtype=ml_dtypes.bfloat16)

    in_maps = []
    for c in range(NCORES):
        xc = x[c * BL:(c + 1) * BL, :T].astype(np.int64)  # (BL, T)
        oneh = np.zeros((VOCAB, T, BL), dtype=ml_dtypes.bfloat16)
        tt, bb = np.meshgrid(np.arange(T), np.arange(BL), indexing="ij")
        oneh[xc.T[tt, bb], tt, bb] = 1
        in_maps.append({
            "oneh": np.ascontiguousarray(oneh).reshape(VOCAB, T * BL),
            "g": g_bf,
            "whh": whh_bf,
            "linw": linw_bf,
            "ident": ident,
        })
    return in_maps


_NC_CACHE = {}


def run_cores(x, embed_W, Wxh, Whh, bh, lin_W, T=T_FULL, trace=False):
    if T not in _NC_CACHE:
        _NC_CACHE[T] = build_nc(T)
    nc = _NC_CACHE[T]
    in_maps = prep_inputs(x, embed_W, Wxh, Whh, bh, lin_W, T=T)
    res = run_bass_kernel_spmd(nc, in_maps, list(range(NCORES)), trace=trace)
    logits = np.empty((B_FULL, T, VOCAB), np.float32)
    h_last = np.empty((B_FULL, HIDDEN), np.float32)
    for c in range(NCORES):
        lt = res.results[c]["logitsT"].reshape(VOCAB, T, BL)
        logits[c * BL:(c + 1) * BL] = lt.transpose(2, 1, 0)
        h_last[c * BL:(c + 1) * BL] = res.results[c]["hlast"]
    return logits, h_last, res


def kernel(x, embed_W, Wxh, Whh, bh, lin_W, lin_b):
    logits, h_last, _ = run_cores(x, embed_W, Wxh, Whh, bh, lin_W, T=T_FULL)
    logits = logits + np.asarray(lin_b, np.float32)[None, None, :]
    return logits, h_last
